# revision 1
# baseline (speedup 1.0000x reference)
"""Causal single-head attention (B=4, T=4096, D=1024, H=64) on 8 TRN2 cores.

Sharding: 2 cores per batch, queries split zig-zag for causal load balance:
  half0 (cores 0-3):  query rows [0,1024) u [3072,4096), keys all [0,4096)
  half1 (cores 4-7):  query rows [1024,3072),             keys     [0,3072)

One SPMD program; phase 1 (projection) is shared, phase 2 (attention) is
specialized per half via a partition-id If/Else with exact causal chunk
schedules (no padding, no kill machinery).

Compute layout: scores are built transposed (s on partitions, t free) so
both attention matmuls take projection outputs directly:
  scores^T[s,t] = sum_h kT[h,s] qT[h,t]   (lhsT = kT chunk, rhs = qT slot)
  out^T[h|1, t] = sum_s v_aug[s, h|1] p^T[s, t]
Softmax runs without max-subtraction (scores bounded ~+-2.5); the
denominator comes from an appended ones-column on V (PV matmul m=65).
k^T and q^T are duplicated on partitions 64:128 so score matmuls can be
row-packed in concurrent pairs (tile_position rows 0/64).
"""

import numpy as np

import concourse.bass as bass
import concourse.mybir as mybir
from concourse import bacc
from concourse.tile import TileContext
from concourse.masks import make_identity
from concourse.bass_utils import run_bass_kernel_spmd

B, T, D, H = 4, 4096, 1024, 64
NCORES = 8
NQ = 2048
SCALE = 1.0 / np.sqrt(D)  # 1/32

# per-half schedules: list of (out_slot, q_tile, [chunk list, diag first])
def _sched(half):
    tiles = [0, 1, 6, 7] if half == 0 else [2, 3, 4, 5]
    out = []
    for slot, t in enumerate(tiles):
        diag = [4 * t + i for i in range(4)]
        fills = list(range(0, 4 * t))
        out.append((slot, t, diag + fills))
    return out

_CACHE = {}


def _build():
    if "nc" in _CACHE:
        return _CACHE["nc"]
    f32 = mybir.dt.float32
    bf16 = mybir.dt.bfloat16
    AF = mybir.ActivationFunctionType

    nc = bacc.Bacc(None, target_bir_lowering=False)
    x_d = nc.declare_dram_parameter("x", [T, D], f32, isOutput=False)
    wkq_d = nc.declare_dram_parameter("wkq", [D, 128], f32, isOutput=False)
    wv_d = nc.declare_dram_parameter("wv", [D, H], f32, isOutput=False)
    out_d = nc.declare_dram_parameter("out", [NQ, H], f32, isOutput=True)

    with TileContext(nc) as tc:
        with (
            tc.tile_pool(name="persist", bufs=1) as pp,
            tc.tile_pool(name="work", bufs=2) as pw,
        ):
            # ---- phase 1 data (issue x group-0/1 DMAs before anything else) ----
            xbs = []
            for g in range(2):
                xb = pw.tile([128, 4096], bf16, tag="xb")
                for h in range(2):  # two DMAs per group: d-cols 0:512 / 512:1024
                    nc.gpsimd.dma_start(
                        out=xb[:, :].rearrange("p (c d) -> p c d", d=1024)[:, :, 512 * h: 512 * (h + 1)],
                        in_=x_d[512 * g: 512 * (g + 1), :].rearrange(
                            "(c p) d -> p c d", p=128)[:, :, 512 * h: 512 * (h + 1)])
                xbs.append(xb)

            # ---- constants ----
            ident_f = pp.tile([128, 128], f32, tag="idf")
            make_identity(nc, ident_f[:, :])
            ident_b = pp.tile([128, 128], bf16, tag="idb")
            nc.vector.tensor_copy(ident_b[:, :], ident_f[:, :])

            # mask_big[p, g] = 1 iff g >= p + 384 (else 0)
            mask_f = pp.tile([128, 896], f32, tag="mkf")
            nc.gpsimd.memset(mask_f[:, :], 0.0)
            nc.gpsimd.affine_select(
                out=mask_f[:, :], in_=mask_f[:, :],
                compare_op=mybir.AluOpType.is_gt, fill=1.0,
                base=384, pattern=[[-1, 896]], channel_multiplier=1,
            )
            mask_b = pp.tile([128, 896], bf16, tag="mkb")
            nc.vector.tensor_copy(mask_b[:, :], mask_f[:, :])

            # weights (cast f32->bf16 during DMA; [Wk | Wq] packed)
            wkq = pp.tile([128, 1024], bf16, tag="wkq")
            nc.gpsimd.dma_start(
                out=wkq[:, :].rearrange("p (c h) -> p c h", h=128),
                in_=wkq_d[:, :].rearrange("(c p) h -> p c h", p=128))
            wv = pp.tile([128, 512], bf16, tag="wv")
            nc.gpsimd.dma_start(
                out=wv[:, :].rearrange("p (c h) -> p c h", h=64),
                in_=wv_d[:, :].rearrange("(c p) h -> p c h", p=128))

            # persistent activations
            kT = pp.tile([128, T], bf16, tag="kT")     # k^T; rows 64:128 dup
            qT = pp.tile([128, T], bf16, tag="qT")     # q^T; rows 0:64 dup
            qkstage = pp.tile([128, T], bf16, tag="qkstage")
            vsb = pp.tile([128, 32 * 65], bf16, tag="vsb")
            nc.vector.memset(vsb[:, :], 1.0)           # col 64 of each chunk = 1

            # ---- phase 1: transpose + project, 8 groups of 512 rows ----
            with tc.tile_pool(name="ps1", bufs=1, space="PSUM") as ps1:
                for g in range(8):
                    if g >= 2:
                        xb = pw.tile([128, 4096], bf16, tag="xb")
                        for h in range(2):
                            nc.gpsimd.dma_start(
                                out=xb[:, :].rearrange("p (c d) -> p c d", d=1024)[:, :, 512 * h: 512 * (h + 1)],
                                in_=x_d[512 * g: 512 * (g + 1), :].rearrange(
                                    "(c p) d -> p c d", p=128)[:, :, 512 * h: 512 * (h + 1)])
                    else:
                        xb = xbs[g]
                    xT = pw.tile([128, 4096], bf16, tag="xT")
                    for dc in range(8):
                        pxt = ps1.tile([128, 512], f32, tag=f"xt{dc % 4}")
                        for c in range(4):
                            nc.tensor.matmul(
                                pxt[:, 128 * c: 128 * (c + 1)],
                                lhsT=xb[:, 1024 * c + 128 * dc: 1024 * c + 128 * (dc + 1)],
                                rhs=ident_b[:, :], start=True, stop=True)
                        if dc % 2 == 0:
                            nc.scalar.copy(xT[:, 512 * dc: 512 * (dc + 1)], pxt[:, :])
                        else:
                            nc.vector.tensor_copy(xT[:, 512 * dc: 512 * (dc + 1)], pxt[:, :])

                    pqk = ps1.tile([128, 512], f32, tag="qk")
                    for dc in range(8):
                        nc.tensor.matmul(
                            pqk[:, :], lhsT=wkq[:, 128 * dc: 128 * (dc + 1)],
                            rhs=xT[:, 512 * dc: 512 * (dc + 1)],
                            start=(dc == 0), stop=(dc == 7))
                    # k rows 0:64 -> kT direct; q rows 64:128 -> qT direct;
                    # the opposite partition halves come via SBUF->SBUF DMA.
                    nc.scalar.copy(kT[0:64, 512 * g: 512 * (g + 1)], pqk[0:64, :])
                    nc.vector.tensor_copy(qT[64:128, 512 * g: 512 * (g + 1)], pqk[64:128, :])
                    nc.scalar.copy(qkstage[0:64, 512 * g: 512 * (g + 1)], pqk[0:64, :])
                    nc.vector.tensor_copy(qkstage[64:128, 512 * g: 512 * (g + 1)], pqk[64:128, :])

                    pv = ps1.tile([64, 512], f32, tag="v")
                    for dc in range(8):
                        nc.tensor.matmul(
                            pv[:, :], lhsT=wv[:, 64 * dc: 64 * (dc + 1)],
                            rhs=xT[:, 512 * dc: 512 * (dc + 1)],
                            start=(dc == 0), stop=(dc == 7))
                    vT = pw.tile([64, 512], bf16, tag="vT")
                    nc.scalar.copy(vT[:, :], pv[:, :])
                    pvn = ps1.tile([128, 256], f32, tag="vn")
                    for c in range(4):
                        nc.tensor.matmul(
                            pvn[:, 64 * c: 64 * (c + 1)],
                            lhsT=vT[0:64, 128 * c: 128 * (c + 1)],
                            rhs=ident_b[0:64, 0:64], start=True, stop=True)
                    nc.vector.tensor_copy(
                        vsb[:, 65 * 4 * g: 65 * 4 * (g + 1)].rearrange(
                            "p (c h) -> p c h", h=65)[:, :, 0:64],
                        pvn[:, :].rearrange("p (c h) -> p c h", h=64))


            # partition shifts: kT dup to rows 64:128, q to rows 0:64
            for hg in range(2):
                csl = slice(2048 * hg, 2048 * (hg + 1))
                nc.gpsimd.dma_start(out=kT[64:128, csl], in_=qkstage[0:64, csl])
                nc.gpsimd.dma_start(out=qT[0:64, csl], in_=qkstage[64:128, csl])

            # ---- phase 2: attention, specialized per half ----
            with (
                tc.tile_pool(name="ps2", bufs=1, space="PSUM") as ps2,
                tc.tile_pool(name="ps3", bufs=1, space="PSUM") as ps3,
            ):
                # all tiles pre-allocated OUTSIDE the If (pool allocation
                # inside conditional branches breaks Tile's wait assignment)
                ps_bufs = [ps2.tile([128, 1536], f32, tag=f"sc{i}", name=f"scb{i}") for i in range(2)]
                pT_bufs = [pw.tile([128, 1536], bf16, tag=f"pT{i}", name=f"pTb{i}") for i in range(2)]
                po = ps3.tile([65, 512], f32, tag="po")
                pe2 = ps3.tile([128, 260], f32, tag="epi")
                osb = pw.tile([65, 512], f32, tag="osb")
                rc = pw.tile([128, 4], f32, tag="rc")
                outsb = pw.tile([128, 1024], f32, tag="outsb")

                def phase2(half):
                    gi = 0
                    for slot, qt, chunks in _sched(half):
                        n = len(chunks)
                        tsl = slice(512 * qt, 512 * (qt + 1))
                        pos = 0
                        while pos < n:
                            grp = chunks[pos: pos + 3]
                            w = 512 * len(grp)
                            ps = ps_bufs[gi % 2]
                            pT = pT_bufs[gi % 2]
                            gi += 1
                            jj = 0
                            while jj < len(grp):
                                if jj + 1 < len(grp):  # row-packed pair
                                    c0, c1 = grp[jj], grp[jj + 1]
                                    nc.tensor.matmul(
                                        ps[:, 512 * jj: 512 * (jj + 1)],
                                        lhsT=kT[0:64, 128 * c0: 128 * (c0 + 1)],
                                        rhs=qT[0:64, tsl], start=True, stop=True)
                                    nc.tensor.matmul(
                                        ps[:, 512 * (jj + 1): 512 * (jj + 2)],
                                        lhsT=kT[64:128, 128 * c1: 128 * (c1 + 1)],
                                        rhs=qT[64:128, tsl], start=True, stop=True)
                                    jj += 2
                                else:
                                    c0 = grp[jj]
                                    nc.tensor.matmul(
                                        ps[:, 512 * jj: 512 * (jj + 1)],
                                        lhsT=kT[0:64, 128 * c0: 128 * (c0 + 1)],
                                        rhs=qT[0:64, tsl], start=True, stop=True)
                                    jj += 1
                            nc.scalar.activation(pT[:, 0:w], ps[:, 0:w], AF.Exp, scale=SCALE)
                            for jj, ch in enumerate(grp):
                                p = pos + jj
                                if p < 4:  # diagonal chunk: causal mask
                                    delta = 128 * p
                                    nc.vector.tensor_mul(
                                        pT[:, 512 * jj: 512 * (jj + 1)],
                                        pT[:, 512 * jj: 512 * (jj + 1)],
                                        mask_b[:, 384 - delta: 896 - delta])
                                nc.tensor.matmul(
                                    po[:, :], lhsT=vsb[:, 65 * ch: 65 * ch + 65],
                                    rhs=pT[:, 512 * jj: 512 * (jj + 1)],
                                    start=(p == 0), stop=(p == n - 1))
                            pos += len(grp)

                        # epilogue: transpose [65,512] -> [512,65], divide, store
                        nc.vector.tensor_copy(osb[:, :], po[:, :])
                        for c in range(4):
                            nc.tensor.matmul(
                                pe2[:, 65 * c: 65 * (c + 1)],
                                lhsT=osb[0:65, 128 * c: 128 * (c + 1)],
                                rhs=ident_f[0:65, 0:65], start=True, stop=True)
                        for c in range(4):
                            nc.vector.reciprocal(rc[:, c: c + 1], pe2[:, 65 * c + 64: 65 * c + 65])
                            nc.vector.tensor_scalar_mul(
                                outsb[:, 256 * slot + 64 * c: 256 * slot + 64 * (c + 1)],
                                pe2[:, 65 * c: 65 * c + 64], rc[:, c: c + 1])

                pid = nc.partition_id(engines=[
                    mybir.EngineType.PE, mybir.EngineType.Activation,
                    mybir.EngineType.DVE])
                with tc.If(pid < 4) as cmp:
                    phase2(0)
                with cmp.Else():
                    phase2(1)

                # store all four slots after the If (no DMAs inside branches)
                nc.sync.dma_start(
                    out=out_d[:, :].rearrange("(s c p) h -> p s c h", p=128, c=4),
                    in_=outsb[:, :].rearrange("p (s c h) -> p s c h", h=64, c=4))

    nc.compile()
    _CACHE["nc"] = nc
    return nc


def _in_maps(x, Wq, Wk, Wv):
    f4 = np.float32
    wkq = np.concatenate([Wk, Wq], axis=1).astype(f4)   # [D, 128], k first
    wv = np.asarray(Wv, f4)
    maps = []
    for c in range(NCORES):
        b = c % 4
        maps.append({"x": np.ascontiguousarray(np.asarray(x[b], f4)),
                     "wkq": wkq, "wv": wv})
    return maps


def _install_profile_shim():
    import sys, types
    import concourse.bass_utils as bu
    bu.upload_artifacts = lambda tmpdir: "local://" + tmpdir
    if "antenv.axon_hooks" in sys.modules:
        return
    mod = types.ModuleType("antenv.axon_hooks")
    holder = []
    mod.set_axon_ntff_profile_hook = holder.append
    mod.get_axon_ntff_profile_hook = lambda: holder[-1] if holder else None
    sys.modules["antenv.axon_hooks"] = mod
    import antenv
    antenv.axon_hooks = mod
    from trn_agent_boot.trn_boot import _ntff_profile_via_ctypes
    mod.set_axon_ntff_profile_hook(_ntff_profile_via_ctypes("/opt/axon/libaxon_pjrt.so"))


def kernel(x, Wq, Wk, Wv, _want_profile=False):
    if _want_profile:
        _install_profile_shim()
    nc = _build()
    maps = _in_maps(x, Wq, Wk, Wv)
    res = run_bass_kernel_spmd(nc, maps, core_ids=list(range(NCORES)),
                               trace=_want_profile)
    out = np.empty((B, T, H), np.float32)
    for c in range(NCORES):
        b, half = c % 4, c // 4
        r = np.asarray(res.results[c]["out"])
        if half == 0:
            out[b, 0:1024] = r[0:1024]
            out[b, 3072:4096] = r[1024:2048]
        else:
            out[b, 1024:3072] = r
    if _want_profile:
        return out, res
    return out



# revision 4
# speedup vs baseline: 1.1385x; 1.1385x over previous
"""Causal single-head attention (B=4, T=4096, D=1024, H=64) on 8 TRN2 cores.

Sharding: 2 cores per batch, queries split zig-zag for causal load balance:
  half0 (cores 0-3):  query rows [0,1024) u [3072,4096), keys all [0,4096)
  half1 (cores 4-7):  query rows [1024,3072),             keys     [0,3072)

The host pre-transposes and bf16-casts x per core (x^T [D, T]) so the
kernel needs NO on-device transpose of x: projections read x^T directly
(d on partitions).  Weights are host-packed into SBUF partition layout.
HBM traffic per core: 8.4MB (bf16) instead of 16.8MB (f32).

Phase 1 (uniform, outside the If): for each 512-row group g:
  pqk[128, 512] = [Wk|Wq]^T x^T   (8 accumulating matmuls over d-chunks)
  pv[64, 512]   = Wv^T x^T        -> vT -> PE-transpose -> vsb [s, h|1]
  kT/qT partition dup via SBUF-SBUF DMA at the end (for row-packed scores).

Phase 2 (If/Else on partition id, engines PE/Act/DVE only):
  scores^T[s,t] built per 128-key chunk (row-packed concurrent pairs),
  exp on Act engine (no max subtraction; scores*scale bounded ~+-2.5),
  causal mask multiply on DVE for diagonal chunks,
  PV accumulation with ones-column (m=65) giving the denominator,
  epilogue: DVE reciprocal of denom row + 1-row broadcast matmul + DVE
  multiply; output stays transposed [64, 2048] and the host transposes.
"""

import numpy as np
import ml_dtypes

import concourse.bass as bass
import concourse.mybir as mybir
from concourse import bacc
from concourse.tile import TileContext
from concourse.masks import make_identity
from concourse.bass_utils import run_bass_kernel_spmd

B, T, D, H = 4, 4096, 1024, 64
NCORES = 8
NQ = 2048
SCALE = 1.0 / np.sqrt(D)  # 1/32
BF16 = ml_dtypes.bfloat16

# per-half schedules: list of (out_slot, q_tile, [chunk list, diag first])
def _sched(half):
    tiles = [0, 1, 6, 7] if half == 0 else [2, 3, 4, 5]
    out = []
    for slot, t in enumerate(tiles):
        diag = [4 * t + i for i in range(4)]
        fills = list(range(0, 4 * t))
        out.append((slot, t, diag + fills))
    return out

_CACHE = {}


def _build():
    if "nc" in _CACHE:
        return _CACHE["nc"]
    f32 = mybir.dt.float32
    bf16 = mybir.dt.bfloat16
    AF = mybir.ActivationFunctionType

    nc = bacc.Bacc(None, target_bir_lowering=False)
    x_d = nc.declare_dram_parameter("xt", [D, T], bf16, isOutput=False)
    wkq_d = nc.declare_dram_parameter("wkq", [128, 1024], bf16, isOutput=False)
    wv_d = nc.declare_dram_parameter("wv", [128, 512], bf16, isOutput=False)
    out_d = nc.declare_dram_parameter("out", [H, NQ], f32, isOutput=True)

    with TileContext(nc) as tc:
        with (
            tc.tile_pool(name="persist", bufs=1) as pp,
            tc.tile_pool(name="work", bufs=2) as pw,
        ):
            # ---- weights first (tiny, gate the first projection) ----
            wkq = pp.tile([128, 1024], bf16, tag="wkq")
            nc.gpsimd.dma_start(out=wkq[:, :], in_=wkq_d[:, :])
            wv = pp.tile([128, 512], bf16, tag="wv")
            nc.gpsimd.dma_start(out=wv[:, :], in_=wv_d[:, :])

            # ---- x^T loads: one DMA per 512-column t-group ----
            xsb = pp.tile([128, 8 * T], bf16, tag="xsb")  # [p, (dc, t)]
            xview = xsb[:, :].rearrange("p (c t) -> p c t", t=T)
            xdram = x_d[:, :].rearrange("(c p) t -> p c t", p=128)
            for g in range(8):
                eng = nc.gpsimd if g < 4 else nc.sync
                eng.dma_start(
                    out=xview[:, :, 512 * g: 512 * (g + 1)],
                    in_=xdram[:, :, 512 * g: 512 * (g + 1)])

            # ---- constants ----
            ident_f = pp.tile([128, 128], f32, tag="idf")
            make_identity(nc, ident_f[:, :])
            ident_b = pp.tile([128, 128], bf16, tag="idb")
            nc.vector.tensor_copy(ident_b[:, :], ident_f[:, :])

            # mask_big[p, g] = 1 iff g >= p + 384 (else 0)
            mask_f = pp.tile([128, 896], f32, tag="mkf")
            nc.gpsimd.memset(mask_f[:, :], 0.0)
            nc.gpsimd.affine_select(
                out=mask_f[:, :], in_=mask_f[:, :],
                compare_op=mybir.AluOpType.is_gt, fill=1.0,
                base=384, pattern=[[-1, 896]], channel_multiplier=1,
            )
            mask_b = pp.tile([128, 896], bf16, tag="mkb")
            nc.vector.tensor_copy(mask_b[:, :], mask_f[:, :])

            ones_b = pp.tile([128, 64], bf16, tag="ones")
            nc.vector.memset(ones_b[:, :], 1.0)

            # persistent activations
            kT = pp.tile([128, T], bf16, tag="kT")     # k^T; rows 64:128 dup
            qT = pp.tile([128, T], bf16, tag="qT")     # q^T; rows 0:64 dup
            vsb = pp.tile([128, 32 * 65], bf16, tag="vsb")
            nc.vector.memset(vsb[:, :], 1.0)           # col 64 of each chunk = 1

            # preload the exp activation table early (hide the ~1.3us load)
            warm = pw.tile([1, 1], f32, tag="warm")
            nc.scalar.activation(warm[:, :], ones_b[0:1, 0:1], AF.Exp, scale=1.0)

            # ---- phase 1: project, 8 groups of 512 t-columns ----
            with tc.tile_pool(name="ps1", bufs=2, space="PSUM") as ps1:
                for g in range(8):
                    tsl = slice(512 * g, 512 * (g + 1))
                    pqk = ps1.tile([128, 512], f32, tag="pqk")
                    for dc in range(8):
                        nc.tensor.matmul(
                            pqk[:, :], lhsT=wkq[:, 128 * dc: 128 * (dc + 1)],
                            rhs=xsb[:, 4096 * dc + 512 * g: 4096 * dc + 512 * (g + 1)],
                            start=(dc == 0), stop=(dc == 7))
                    nc.scalar.copy(kT[0:64, tsl], pqk[0:64, :])
                    nc.vector.tensor_copy(qT[64:128, tsl], pqk[64:128, :])

                    pv = ps1.tile([64, 512], f32, tag="pv")
                    for dc in range(8):
                        nc.tensor.matmul(
                            pv[:, :], lhsT=wv[:, 64 * dc: 64 * (dc + 1)],
                            rhs=xsb[:, 4096 * dc + 512 * g: 4096 * dc + 512 * (g + 1)],
                            start=(dc == 0), stop=(dc == 7))
                    vT = pw.tile([64, 512], bf16, tag="vT")
                    nc.scalar.copy(vT[:, :], pv[:, :])
                    pvn = ps1.tile([128, 256], f32, tag="pvn")
                    for c in range(4):
                        nc.tensor.matmul(
                            pvn[:, 64 * c: 64 * (c + 1)],
                            lhsT=vT[0:64, 128 * c: 128 * (c + 1)],
                            rhs=ident_b[0:64, 0:64], start=True, stop=True)
                    nc.vector.tensor_copy(
                        vsb[:, 65 * 4 * g: 65 * 4 * (g + 1)].rearrange(
                            "p (c h) -> p c h", h=65)[:, :, 0:64],
                        pvn[:, :].rearrange("p (c h) -> p c h", h=64))

            # partition dups: kT rows 0:64 -> 64:128, qT rows 64:128 -> 0:64
            for hg in range(2):
                csl = slice(2048 * hg, 2048 * (hg + 1))
                nc.sync.dma_start(out=kT[64:128, csl], in_=kT[0:64, csl])
                nc.sync.dma_start(out=qT[0:64, csl], in_=qT[64:128, csl])

            # ---- phase 2: attention, specialized per half ----
            with (
                tc.tile_pool(name="ps2", bufs=1, space="PSUM") as ps2,
                tc.tile_pool(name="ps3", bufs=1, space="PSUM") as ps3,
            ):
                # all tiles pre-allocated OUTSIDE the If (pool allocation
                # inside conditional branches breaks Tile's wait assignment)
                ps_bufs = [ps2.tile([128, 1536], f32, tag=f"sc{i}", name=f"scb{i}") for i in range(2)]
                pT_bufs = [pw.tile([128, 1536], bf16, tag=f"pT{i}", name=f"pTb{i}") for i in range(2)]
                po = ps3.tile([65, 512], f32, tag="po")
                pbc = ps3.tile([64, 512], f32, tag="pbc")
                rc_bufs = [pw.tile([128, 512], f32, tag=f"rc{i}", name=f"rcb{i}") for i in range(2)]
                rb_bufs = [pw.tile([128, 512], bf16, tag=f"rb{i}", name=f"rbb{i}") for i in range(2)]
                bc_bufs = [pw.tile([64, 512], f32, tag=f"bc{i}", name=f"bcb{i}") for i in range(2)]
                outsb = pw.tile([64, 2048], f32, tag="outsb")

                def phase2(half):
                    gi = 0
                    for slot, qt, chunks in _sched(half):
                        n = len(chunks)
                        tsl = slice(512 * qt, 512 * (qt + 1))
                        pos = 0
                        while pos < n:
                            grp = chunks[pos: pos + 3]
                            w = 512 * len(grp)
                            ps = ps_bufs[gi % 2]
                            pT = pT_bufs[gi % 2]
                            gi += 1
                            jj = 0
                            while jj < len(grp):
                                if jj + 1 < len(grp):  # row-packed pair
                                    c0, c1 = grp[jj], grp[jj + 1]
                                    nc.tensor.matmul(
                                        ps[:, 512 * jj: 512 * (jj + 1)],
                                        lhsT=kT[0:64, 128 * c0: 128 * (c0 + 1)],
                                        rhs=qT[0:64, tsl], start=True, stop=True)
                                    nc.tensor.matmul(
                                        ps[:, 512 * (jj + 1): 512 * (jj + 2)],
                                        lhsT=kT[64:128, 128 * c1: 128 * (c1 + 1)],
                                        rhs=qT[64:128, tsl], start=True, stop=True)
                                    jj += 2
                                else:
                                    c0 = grp[jj]
                                    nc.tensor.matmul(
                                        ps[:, 512 * jj: 512 * (jj + 1)],
                                        lhsT=kT[0:64, 128 * c0: 128 * (c0 + 1)],
                                        rhs=qT[0:64, tsl], start=True, stop=True)
                                    jj += 1
                            nc.scalar.activation(pT[:, 0:w], ps[:, 0:w], AF.Exp, scale=SCALE)
                            for jj, ch in enumerate(grp):
                                p = pos + jj
                                if p < 4:  # diagonal chunk: causal mask
                                    delta = 128 * p
                                    nc.vector.tensor_mul(
                                        pT[:, 512 * jj: 512 * (jj + 1)],
                                        pT[:, 512 * jj: 512 * (jj + 1)],
                                        mask_b[:, 384 - delta: 896 - delta])
                                nc.tensor.matmul(
                                    po[:, :], lhsT=vsb[:, 65 * ch: 65 * ch + 65],
                                    rhs=pT[:, 512 * jj: 512 * (jj + 1)],
                                    start=(p == 0), stop=(p == n - 1))
                            pos += len(grp)

                        # epilogue: normalize in-place, output stays [h, t]
                        rc = rc_bufs[slot % 2]
                        rb = rb_bufs[slot % 2]
                        nc.vector.reciprocal(rc[64:65, :], po[64:65, :])
                        nc.vector.tensor_copy(rb[64:65, :], rc[64:65, :])
                        nc.tensor.matmul(
                            pbc[:, :], lhsT=ones_b[64:65, 0:64],
                            rhs=rb[64:65, :], start=True, stop=True)
                        bc = bc_bufs[slot % 2]
                        nc.scalar.copy(bc[:, :], pbc[:, :])
                        nc.vector.tensor_mul(
                            outsb[:, 512 * slot: 512 * (slot + 1)],
                            po[0:64, :], bc[:, :])

                pid = nc.partition_id(engines=[
                    mybir.EngineType.PE, mybir.EngineType.Activation,
                    mybir.EngineType.DVE])
                with tc.If(pid < 4) as cmp:
                    phase2(0)
                with cmp.Else():
                    phase2(1)

                # store after the If (no DMAs inside branches)
                nc.sync.dma_start(out=out_d[:, :], in_=outsb[:, :])

    nc.compile()
    _CACHE["nc"] = nc
    return nc


def _in_maps(x, Wq, Wk, Wv):
    wkq = np.concatenate([np.asarray(Wk), np.asarray(Wq)], axis=1).astype(BF16)
    wkq_sb = np.ascontiguousarray(
        wkq.reshape(8, 128, 128).transpose(1, 0, 2).reshape(128, 1024))
    wv_sb = np.ascontiguousarray(
        np.asarray(Wv).astype(BF16).reshape(8, 128, 64)
        .transpose(1, 0, 2).reshape(128, 512))
    xts = [np.ascontiguousarray(np.asarray(x[b]).T.astype(BF16)) for b in range(B)]
    maps = []
    for c in range(NCORES):
        b = c % 4
        maps.append({"xt": xts[b], "wkq": wkq_sb, "wv": wv_sb})
    return maps


def _install_profile_shim():
    import sys, types
    import concourse.bass_utils as bu
    bu.upload_artifacts = lambda tmpdir: "local://" + tmpdir
    if "antenv.axon_hooks" in sys.modules:
        return
    mod = types.ModuleType("antenv.axon_hooks")
    holder = []
    mod.set_axon_ntff_profile_hook = holder.append
    mod.get_axon_ntff_profile_hook = lambda: holder[-1] if holder else None
    sys.modules["antenv.axon_hooks"] = mod
    import antenv
    antenv.axon_hooks = mod
    from trn_agent_boot.trn_boot import _ntff_profile_via_ctypes
    mod.set_axon_ntff_profile_hook(_ntff_profile_via_ctypes("/opt/axon/libaxon_pjrt.so"))


def kernel(x, Wq, Wk, Wv, _want_profile=False):
    if _want_profile:
        _install_profile_shim()
    nc = _build()
    maps = _in_maps(x, Wq, Wk, Wv)
    res = run_bass_kernel_spmd(nc, maps, core_ids=list(range(NCORES)),
                               trace=_want_profile)
    out = np.empty((B, T, H), np.float32)
    for c in range(NCORES):
        b, half = c % 4, c // 4
        r = np.asarray(res.results[c]["out"])  # [64, 2048]
        if half == 0:
            out[b, 0:1024] = r[:, 0:1024].T
            out[b, 3072:4096] = r[:, 1024:2048].T
        else:
            out[b, 1024:3072] = r.T
    if _want_profile:
        return out, res
    return out


# revision 9
# speedup vs baseline: 1.1981x; 1.0523x over previous
"""Causal single-head attention (B=4, T=4096, D=1024, H=64) on 8 TRN2 cores.

Sharding: 2 cores per batch, queries split zig-zag for causal load balance:
  half0 (cores 0-3):  query rows [0,1024) u [3072,4096), keys all [0,4096)
  half1 (cores 4-7):  query rows [1024,3072),             keys     [0,3072)

The host pre-transposes and bf16-casts x per core (x^T [D, T]) so the
kernel needs NO on-device transpose of x: projections read x^T directly
(d on partitions).  Weights are host-packed into SBUF partition layout.
HBM traffic per core: 8.4MB (bf16) instead of 16.8MB (f32).

Phase 1 (uniform, outside the If): for each 512-row group g:
  pqk[128, 512] = [Wk|Wq]^T x^T   (8 accumulating matmuls over d-chunks)
  pv[64, 512]   = Wv^T x^T        -> vT -> PE-transpose -> vsb [s, h|1]
  kT/qT partition dup via SBUF-SBUF DMA at the end (for row-packed scores).

Phase 2 (If/Else on partition id, engines PE/Act/DVE only):
  scores^T[s,t] built per 128-key chunk (row-packed concurrent pairs),
  exp on Act engine (no max subtraction; scores*scale bounded ~+-2.5),
  causal mask multiply on DVE for diagonal chunks,
  PV accumulation with ones-column (m=65) giving the denominator,
  epilogue: DVE reciprocal of denom row + 1-row broadcast matmul + DVE
  multiply; output stays transposed [64, 2048] and the host transposes.
"""

import numpy as np
import ml_dtypes

import concourse.bass as bass
import concourse.mybir as mybir
from concourse import bacc
from concourse.tile import TileContext
from concourse.masks import make_identity
from concourse.bass_utils import run_bass_kernel_spmd

B, T, D, H = 4, 4096, 1024, 64
NCORES = 8
NQ = 2048
SCALE = 1.0 / np.sqrt(D)  # 1/32
BF16 = ml_dtypes.bfloat16

# per-half schedules: list of (out_slot, q_tile, [chunk list, diag first])
def _sched(half):
    tiles = [0, 1, 6, 7] if half == 0 else [2, 3, 4, 5]
    out = []
    for slot, t in enumerate(tiles):
        diag = [4 * t + i for i in range(4)]
        fills = list(range(0, 4 * t))
        out.append((slot, t, diag + fills))
    return out

_CACHE = {}


def _build():
    if "nc" in _CACHE:
        return _CACHE["nc"]
    f32 = mybir.dt.float32
    bf16 = mybir.dt.bfloat16
    AF = mybir.ActivationFunctionType

    nc = bacc.Bacc(None, target_bir_lowering=False)
    x_d = nc.declare_dram_parameter("xt", [D, T], bf16, isOutput=False)
    wkq_d = nc.declare_dram_parameter("wkq", [128, 1024], bf16, isOutput=False)
    wv_d = nc.declare_dram_parameter("wv", [128, 512], bf16, isOutput=False)
    out_d = nc.declare_dram_parameter("out", [H, NQ], f32, isOutput=True)

    with TileContext(nc) as tc:
        with (
            tc.tile_pool(name="persist", bufs=1) as pp,
            tc.tile_pool(name="work", bufs=2) as pw,
        ):
            # ---- x^T group 0 + weights first (gate the first projection) ----
            xsb = pp.tile([128, 8 * T], bf16, tag="xsb")  # [p, (dc, t)]
            xview = xsb[:, :].rearrange("p (c t) -> p c t", t=T)
            xdram = x_d[:, :].rearrange("(c p) t -> p c t", p=128)

            def xload(eng, g):
                eng.dma_start(
                    out=xview[:, :, 512 * g: 512 * (g + 1)],
                    in_=xdram[:, :, 512 * g: 512 * (g + 1)])

            xload(nc.gpsimd, 0)
            wkq = pp.tile([128, 1024], bf16, tag="wkq")
            nc.gpsimd.dma_start(out=wkq[:, :], in_=wkq_d[:, :])
            wv = pp.tile([128, 512], bf16, tag="wv")
            nc.gpsimd.dma_start(out=wv[:, :], in_=wv_d[:, :])
            for g in range(1, 8):
                xload(nc.gpsimd if g < 4 else nc.sync, g)

            # ---- constants ----
            ident_f = pp.tile([128, 128], f32, tag="idf")
            make_identity(nc, ident_f[:, :])
            ident_b = pp.tile([128, 128], bf16, tag="idb")
            nc.vector.tensor_copy(ident_b[:, :], ident_f[:, :])

            # mask_big[p, g] = 1 iff g >= p + 384 (else 0)
            mask_f = pp.tile([128, 896], f32, tag="mkf")
            nc.gpsimd.memset(mask_f[:, :], 0.0)
            nc.gpsimd.affine_select(
                out=mask_f[:, :], in_=mask_f[:, :],
                compare_op=mybir.AluOpType.is_gt, fill=1.0,
                base=384, pattern=[[-1, 896]], channel_multiplier=1,
            )
            mask_b = pp.tile([128, 896], bf16, tag="mkb")
            nc.vector.tensor_copy(mask_b[:, :], mask_f[:, :])

            # persistent activations
            kT = pp.tile([128, T], bf16, tag="kT")     # k^T; rows 64:128 dup
            qT = pp.tile([128, T], bf16, tag="qT")     # q^T; rows 0:64 dup
            # vsb chunk ch: cols 0:64 = v rows of key chunk ch, cols 64:128 = 1
            # -> PV matmul (m=128) yields numerator on psum rows 0:64 and the
            #    denominator REPLICATED on rows 64:128 (same cycle count).
            vsb = pp.tile([128, 32 * 128], bf16, tag="vsb")
            nc.vector.memset(vsb[:, :], 1.0)

            # preload the exp activation table early (hide the ~1.3us load)
            warm = pw.tile([1, 1], f32, tag="warm")
            nc.scalar.activation(warm[:, :], mask_b[0:1, 0:1], AF.Exp, scale=1.0)

            # ---- phase 1: project, 8 groups of 512 t-columns ----
            with tc.tile_pool(name="ps1", bufs=2, space="PSUM") as ps1:
                for g in range(8):
                    tsl = slice(512 * g, 512 * (g + 1))
                    pqk = ps1.tile([128, 512], f32, tag="pqk")
                    for dc in range(8):
                        nc.tensor.matmul(
                            pqk[:, :], lhsT=wkq[:, 128 * dc: 128 * (dc + 1)],
                            rhs=xsb[:, 4096 * dc + 512 * g: 4096 * dc + 512 * (g + 1)],
                            start=(dc == 0), stop=(dc == 7))
                    nc.scalar.copy(kT[0:64, tsl], pqk[0:64, :])
                    nc.vector.tensor_copy(qT[64:128, tsl], pqk[64:128, :])

                    pv = ps1.tile([64, 512], f32, tag="pv")
                    for dc in range(8):
                        nc.tensor.matmul(
                            pv[:, :], lhsT=wv[:, 64 * dc: 64 * (dc + 1)],
                            rhs=xsb[:, 4096 * dc + 512 * g: 4096 * dc + 512 * (g + 1)],
                            start=(dc == 0), stop=(dc == 7))
                    vT = pw.tile([64, 512], bf16, tag="vT")
                    nc.scalar.copy(vT[:, :], pv[:, :])
                    pvn = ps1.tile([128, 256], f32, tag="pvn")
                    for c in range(4):
                        nc.tensor.matmul(
                            pvn[:, 64 * c: 64 * (c + 1)],
                            lhsT=vT[0:64, 128 * c: 128 * (c + 1)],
                            rhs=ident_b[0:64, 0:64], start=True, stop=True)
                    nc.vector.tensor_copy(
                        vsb[:, 128 * 4 * g: 128 * 4 * (g + 1)].rearrange(
                            "p (c h) -> p c h", h=128)[:, :, 0:64],
                        pvn[:, :].rearrange("p (c h) -> p c h", h=64))

            # partition dups: kT rows 0:64 -> 64:128, qT rows 64:128 -> 0:64
            for hg in range(2):
                csl = slice(2048 * hg, 2048 * (hg + 1))
                nc.sync.dma_start(out=kT[64:128, csl], in_=kT[0:64, csl])
                nc.sync.dma_start(out=qT[0:64, csl], in_=qT[64:128, csl])

            # ---- phase 2: attention, specialized per half ----
            with (
                tc.tile_pool(name="ps2", bufs=1, space="PSUM") as ps2,
                tc.tile_pool(name="ps3", bufs=1, space="PSUM") as ps3,
            ):
                # all tiles pre-allocated OUTSIDE the If (pool allocation
                # inside conditional branches breaks Tile's wait assignment)
                ps_bufs = [ps2.tile([128, 1024], f32, tag=f"sc{i}", name=f"scb{i}") for i in range(3)]
                pT_bufs = [pw.tile([128, 1024], bf16, tag=f"pT{i}", name=f"pTb{i}") for i in range(3)]
                po = ps3.tile([128, 512], f32, tag="po")
                rc_bufs = [pw.tile([64, 512], f32, tag=f"rc{i}", name=f"rcb{i}") for i in range(2)]
                outsb = pw.tile([64, 2048], f32, tag="outsb")

                def phase2(half):
                    gi = 0
                    for slot, qt, chunks in _sched(half):
                        n = len(chunks)  # always a multiple of 4
                        tsl = slice(512 * qt, 512 * (qt + 1))
                        for pos in range(0, n, 2):
                            c0, c1 = chunks[pos], chunks[pos + 1]
                            ps = ps_bufs[gi % 3]
                            pT = pT_bufs[gi % 3]
                            gi += 1
                            # row-packed concurrent pair
                            nc.tensor.matmul(
                                ps[:, 0:512],
                                lhsT=kT[0:64, 128 * c0: 128 * (c0 + 1)],
                                rhs=qT[0:64, tsl], start=True, stop=True)
                            nc.tensor.matmul(
                                ps[:, 512:1024],
                                lhsT=kT[64:128, 128 * c1: 128 * (c1 + 1)],
                                rhs=qT[64:128, tsl], start=True, stop=True)
                            nc.scalar.activation(pT[:, :], ps[:, :], AF.Exp, scale=SCALE)
                            for jj, ch in enumerate((c0, c1)):
                                p = pos + jj
                                if p < 4:  # diagonal chunk: causal mask
                                    delta = 128 * p
                                    nc.vector.tensor_mul(
                                        pT[:, 512 * jj: 512 * (jj + 1)],
                                        pT[:, 512 * jj: 512 * (jj + 1)],
                                        mask_b[:, 384 - delta: 896 - delta])
                                nc.tensor.matmul(
                                    po[:, :], lhsT=vsb[:, 128 * ch: 128 * (ch + 1)],
                                    rhs=pT[:, 512 * jj: 512 * (jj + 1)],
                                    start=(p == 0), stop=(p == n - 1))

                        # epilogue: denominator arrives replicated on psum rows
                        # 64:128; partition-shifted reciprocal then multiply
                        rc = rc_bufs[slot % 2]
                        nc.vector.reciprocal(rc[0:64, :], po[64:128, :])
                        nc.vector.tensor_mul(
                            outsb[:, 512 * slot: 512 * (slot + 1)],
                            po[0:64, :], rc[0:64, :])

                pid = nc.partition_id(engines=[
                    mybir.EngineType.PE, mybir.EngineType.Activation,
                    mybir.EngineType.DVE])
                with tc.If(pid < 4) as cmp:
                    phase2(0)
                with cmp.Else():
                    phase2(1)

                # store after the If (no DMAs inside branches)
                nc.sync.dma_start(out=out_d[:, :], in_=outsb[:, :])

    nc.compile()
    _CACHE["nc"] = nc
    return nc


def _in_maps(x, Wq, Wk, Wv):
    wkq = np.concatenate([np.asarray(Wk), np.asarray(Wq)], axis=1).astype(BF16)
    wkq_sb = np.ascontiguousarray(
        wkq.reshape(8, 128, 128).transpose(1, 0, 2).reshape(128, 1024))
    wv_sb = np.ascontiguousarray(
        np.asarray(Wv).astype(BF16).reshape(8, 128, 64)
        .transpose(1, 0, 2).reshape(128, 512))
    xts = [np.ascontiguousarray(np.asarray(x[b]).T.astype(BF16)) for b in range(B)]
    maps = []
    for c in range(NCORES):
        b = c % 4
        maps.append({"xt": xts[b], "wkq": wkq_sb, "wv": wv_sb})
    return maps


def _install_profile_shim():
    import sys, types
    import concourse.bass_utils as bu
    bu.upload_artifacts = lambda tmpdir: "local://" + tmpdir
    if "antenv.axon_hooks" in sys.modules:
        return
    mod = types.ModuleType("antenv.axon_hooks")
    holder = []
    mod.set_axon_ntff_profile_hook = holder.append
    mod.get_axon_ntff_profile_hook = lambda: holder[-1] if holder else None
    sys.modules["antenv.axon_hooks"] = mod
    import antenv
    antenv.axon_hooks = mod
    from trn_agent_boot.trn_boot import _ntff_profile_via_ctypes
    mod.set_axon_ntff_profile_hook(_ntff_profile_via_ctypes("/opt/axon/libaxon_pjrt.so"))


def kernel(x, Wq, Wk, Wv, _want_profile=False):
    if _want_profile:
        _install_profile_shim()
    nc = _build()
    maps = _in_maps(x, Wq, Wk, Wv)
    res = run_bass_kernel_spmd(nc, maps, core_ids=list(range(NCORES)),
                               trace=_want_profile)
    out = np.empty((B, T, H), np.float32)
    for c in range(NCORES):
        b, half = c % 4, c // 4
        r = np.asarray(res.results[c]["out"])  # [64, 2048]
        if half == 0:
            out[b, 0:1024] = r[:, 0:1024].T
            out[b, 3072:4096] = r[:, 1024:2048].T
        else:
            out[b, 1024:3072] = r.T
    if _want_profile:
        return out, res
    return out


# revision 16
# speedup vs baseline: 1.2267x; 1.0239x over previous
"""Causal single-head attention (B=4, T=4096, D=1024, H=64) on 8 TRN2 cores.

Sharding: 2 cores per batch, queries split zig-zag for causal load balance:
  half0 (cores 0-3):  query rows [0,1024) u [3072,4096), keys all [0,4096)
  half1 (cores 4-7):  query rows [1024,3072),             keys     [0,3072)

The host pre-transposes and bf16-casts x per core (x^T [D, T]) so the
kernel needs NO on-device transpose of x: projections read x^T directly
(d on partitions).  Weights are host-packed into SBUF partition layout.
HBM traffic per core: 8.4MB (bf16) instead of 16.8MB (f32).

Phase 1 (uniform, outside the If): for each 512-row group g:
  pqk[128, 512] = [Wk|Wq]^T x^T   (8 accumulating matmuls over d-chunks)
  pv[64, 512]   = Wv^T x^T        -> vT -> PE-transpose -> vsb [s, h|1]
  kT/qT partition dup via SBUF-SBUF DMA at the end (for row-packed scores).

Phase 2 (If/Else on partition id, engines PE/Act/DVE only):
  scores^T[s,t] built per 128-key chunk (row-packed concurrent pairs),
  exp on Act engine (no max subtraction; scores*scale bounded ~+-2.5),
  causal mask multiply on DVE for diagonal chunks,
  PV accumulation with ones-column (m=65) giving the denominator,
  epilogue: DVE reciprocal of denom row + 1-row broadcast matmul + DVE
  multiply; output stays transposed [64, 2048] and the host transposes.
"""

import numpy as np
import ml_dtypes

import concourse.bass as bass
import concourse.mybir as mybir
from concourse import bacc
from concourse.tile import TileContext
from concourse.masks import make_identity
from concourse.bass_utils import run_bass_kernel_spmd

B, T, D, H = 4, 4096, 1024, 64
NCORES = 8
NQ = 2048
SCALE = 1.0 / np.sqrt(D)  # 1/32
BF16 = ml_dtypes.bfloat16

# per-half schedules: list of (out_slot, q_tile, [chunk list, diag first])
def _sched(half):
    tiles = [0, 1, 6, 7] if half == 0 else [2, 3, 4, 5]
    out = []
    for slot, t in enumerate(tiles):
        diag = [4 * t + i for i in range(4)]
        fills = list(range(0, 4 * t))
        out.append((slot, t, diag + fills))
    return out

_CACHE = {}


def _build():
    if "nc" in _CACHE:
        return _CACHE["nc"]
    f32 = mybir.dt.float32
    bf16 = mybir.dt.bfloat16
    AF = mybir.ActivationFunctionType

    nc = bacc.Bacc(None, target_bir_lowering=False)
    # x^T in group-major layout: x_d[g, p, c*512+t'] = x[512g+t', 128c+p]
    # -> per-(g,p) the DMA reads one contiguous 8KB run
    x_d = nc.declare_dram_parameter("xt", [8, 128, 4096], bf16, isOutput=False)
    wkq_d = nc.declare_dram_parameter("wkq", [128, 1024], bf16, isOutput=False)
    wv_d = nc.declare_dram_parameter("wv", [128, 512], bf16, isOutput=False)
    out_d = nc.declare_dram_parameter("out", [H, NQ], f32, isOutput=True)

    with TileContext(nc) as tc:
        with (
            tc.tile_pool(name="persist", bufs=1) as pp,
            tc.tile_pool(name="work", bufs=2) as pw,
        ):
            # ---- weights (sync) + x^T groups (group 0 first) ----
            wkq = pp.tile([128, 1024], bf16, tag="wkq")
            nc.sync.dma_start(out=wkq[:, :], in_=wkq_d[:, :])
            wv = pp.tile([128, 512], bf16, tag="wv")
            nc.sync.dma_start(out=wv[:, :], in_=wv_d[:, :])

            xsb = pp.tile([128, 8 * T], bf16, tag="xsb")  # [p, (dc, t)]
            xview = xsb[:, :].rearrange("p (c t) -> p c t", t=T)

            def xload(eng, g):
                eng.dma_start(
                    out=xview[:, :, 512 * g: 512 * (g + 1)],
                    in_=x_d[g, :, :].rearrange("p (c t) -> p c t", t=512))

            for g in range(8):
                xload(nc.gpsimd if g < 4 else nc.sync, g)

            # ---- constants ----
            ident_f = pp.tile([128, 128], f32, tag="idf")
            make_identity(nc, ident_f[:, :])
            ident_b = pp.tile([128, 128], bf16, tag="idb")
            nc.vector.tensor_copy(ident_b[:, :], ident_f[:, :])

            # mask_big[p, g] = 1 iff g >= p + 384 (else 0)
            mask_f = pp.tile([128, 896], f32, tag="mkf")
            nc.gpsimd.memset(mask_f[:, :], 0.0)
            nc.gpsimd.affine_select(
                out=mask_f[:, :], in_=mask_f[:, :],
                compare_op=mybir.AluOpType.is_gt, fill=1.0,
                base=384, pattern=[[-1, 896]], channel_multiplier=1,
            )
            mask_b = pp.tile([128, 896], bf16, tag="mkb")
            nc.vector.tensor_copy(mask_b[:, :], mask_f[:, :])

            # persistent activations
            kT = pp.tile([128, T], bf16, tag="kT")     # k^T; rows 64:128 dup
            qT = pp.tile([128, T], bf16, tag="qT")     # q^T; rows 0:64 dup
            # vsb chunk ch: cols 0:64 = v rows of key chunk ch, cols 64:128 = 1
            # -> PV matmul (m=128) yields numerator on psum rows 0:64 and the
            #    denominator REPLICATED on rows 64:128 (same cycle count).
            vsb = pp.tile([128, 32 * 128], bf16, tag="vsb")
            nc.vector.memset(vsb[:, :], 1.0)

            # preload the exp activation table early (hide the ~1.3us load)
            warm = pw.tile([1, 1], f32, tag="warm")
            nc.scalar.activation(warm[:, :], mask_b[0:1, 0:1], AF.Exp, scale=1.0)

            # ---- phase 1: project, 8 groups of 512 t-columns ----
            with tc.tile_pool(name="ps1", bufs=2, space="PSUM") as ps1:
                for g in range(8):
                    tsl = slice(512 * g, 512 * (g + 1))
                    pqk = ps1.tile([128, 512], f32, tag="pqk")
                    for dc in range(8):
                        nc.tensor.matmul(
                            pqk[:, :], lhsT=wkq[:, 128 * dc: 128 * (dc + 1)],
                            rhs=xsb[:, 4096 * dc + 512 * g: 4096 * dc + 512 * (g + 1)],
                            start=(dc == 0), stop=(dc == 7))
                    nc.scalar.copy(kT[0:64, tsl], pqk[0:64, :])
                    nc.vector.tensor_copy(qT[64:128, tsl], pqk[64:128, :])

                    pv = ps1.tile([64, 512], f32, tag="pv")
                    for dc in range(8):
                        nc.tensor.matmul(
                            pv[:, :], lhsT=wv[:, 64 * dc: 64 * (dc + 1)],
                            rhs=xsb[:, 4096 * dc + 512 * g: 4096 * dc + 512 * (g + 1)],
                            start=(dc == 0), stop=(dc == 7))
                    vT = pw.tile([64, 512], bf16, tag="vT")
                    nc.scalar.copy(vT[:, :], pv[:, :])
                    pvn = ps1.tile([128, 256], f32, tag="pvn")
                    for c in range(4):
                        nc.tensor.matmul(
                            pvn[:, 64 * c: 64 * (c + 1)],
                            lhsT=vT[0:64, 128 * c: 128 * (c + 1)],
                            rhs=ident_b[0:64, 0:64], start=True, stop=True)
                    nc.vector.tensor_copy(
                        vsb[:, 128 * 4 * g: 128 * 4 * (g + 1)].rearrange(
                            "p (c h) -> p c h", h=128)[:, :, 0:64],
                        pvn[:, :].rearrange("p (c h) -> p c h", h=64))

            # partition dups: kT rows 0:64 -> 64:128, qT rows 64:128 -> 0:64
            for hg in range(2):
                csl = slice(2048 * hg, 2048 * (hg + 1))
                nc.sync.dma_start(out=kT[64:128, csl], in_=kT[0:64, csl])
                nc.sync.dma_start(out=qT[0:64, csl], in_=qT[64:128, csl])

            # ---- phase 2: attention, specialized per half ----
            with (
                tc.tile_pool(name="ps2", bufs=1, space="PSUM") as ps2,
                tc.tile_pool(name="ps3", bufs=1, space="PSUM") as ps3,
            ):
                # all tiles pre-allocated OUTSIDE the If (pool allocation
                # inside conditional branches breaks Tile's wait assignment)
                ps_bufs = [ps2.tile([128, 1024], f32, tag=f"sc{i}", name=f"scb{i}") for i in range(3)]
                pT_bufs = [pw.tile([128, 1024], bf16, tag=f"pT{i}", name=f"pTb{i}") for i in range(3)]
                po_bufs = [ps3.tile([128, 512], f32, tag=f"po{i}", name=f"pob{i}") for i in range(2)]
                rc_bufs = [pw.tile([128, 512], f32, tag=f"rc{i}", name=f"rcb{i}") for i in range(2)]
                outsb = pw.tile([64, 2048], f32, tag="outsb")

                def phase2(half):
                    # flat pair list: (slot, qt, c0, c1, pos, n)
                    pairs = []
                    for slot, qt, chunks in _sched(half):
                        n = len(chunks)  # multiple of 4
                        for pos in range(0, n, 2):
                            pairs.append((slot, qt, chunks[pos], chunks[pos + 1], pos, n))

                    def emit_scores(i):
                        slot, qt, c0, c1, pos, n = pairs[i]
                        tsl = slice(512 * qt, 512 * (qt + 1))
                        ps = ps_bufs[i % 3]
                        nc.tensor.matmul(
                            ps[:, 0:512],
                            lhsT=kT[0:64, 128 * c0: 128 * (c0 + 1)],
                            rhs=qT[0:64, tsl], start=True, stop=True)
                        nc.tensor.matmul(
                            ps[:, 512:1024],
                            lhsT=kT[64:128, 128 * c1: 128 * (c1 + 1)],
                            rhs=qT[64:128, tsl], start=True, stop=True)

                    emit_scores(0)
                    for i, (slot, qt, c0, c1, pos, n) in enumerate(pairs):
                        ps = ps_bufs[i % 3]
                        pT = pT_bufs[i % 3]
                        po = po_bufs[slot % 2]
                        nc.scalar.activation(pT[:, :], ps[:, :], AF.Exp, scale=SCALE)
                        # keep PE busy during exp(i): emit next pair's scores
                        if i + 1 < len(pairs):
                            emit_scores(i + 1)
                        for jj, ch in enumerate((c0, c1)):
                            p = pos + jj
                            if p < 4:  # diagonal chunk: causal mask
                                delta = 128 * p
                                nc.vector.tensor_mul(
                                    pT[:, 512 * jj: 512 * (jj + 1)],
                                    pT[:, 512 * jj: 512 * (jj + 1)],
                                    mask_b[:, 384 - delta: 896 - delta])
                            nc.tensor.matmul(
                                po[:, :], lhsT=vsb[:, 128 * ch: 128 * (ch + 1)],
                                rhs=pT[:, 512 * jj: 512 * (jj + 1)],
                                start=(p == 0), stop=(p == n - 1))
                        if pos + 2 == n:
                            # epilogue: denominator arrives replicated on psum
                            # rows 64:128; shifted reciprocal then multiply
                            rc = rc_bufs[slot % 2]
                            nc.vector.reciprocal(rc[0:64, :], po[64:128, :])
                            nc.vector.tensor_mul(
                                outsb[:, 512 * slot: 512 * (slot + 1)],
                                po[0:64, :], rc[0:64, :])

                pid = nc.partition_id(engines=[
                    mybir.EngineType.PE, mybir.EngineType.Activation,
                    mybir.EngineType.DVE])
                with tc.If(pid < 4) as cmp:
                    phase2(0)
                with cmp.Else():
                    phase2(1)

                # store after the If (no DMAs inside branches)
                nc.sync.dma_start(out=out_d[:, :], in_=outsb[:, :])

    nc.compile()
    _CACHE["nc"] = nc
    return nc


def _in_maps(x, Wq, Wk, Wv):
    wkq = np.concatenate([np.asarray(Wk), np.asarray(Wq)], axis=1).astype(BF16)
    wkq_sb = np.ascontiguousarray(
        wkq.reshape(8, 128, 128).transpose(1, 0, 2).reshape(128, 1024))
    wv_sb = np.ascontiguousarray(
        np.asarray(Wv).astype(BF16).reshape(8, 128, 64)
        .transpose(1, 0, 2).reshape(128, 512))
    # X5[g, p, c, t'] = x[512g+t', 128c+p]  (8KB contiguous per (g, p))
    xts = [np.ascontiguousarray(
        np.asarray(x[b]).astype(BF16).reshape(8, 512, 8, 128)
        .transpose(0, 3, 2, 1).reshape(8, 128, 4096)) for b in range(B)]
    maps = []
    for c in range(NCORES):
        b = c % 4
        maps.append({"xt": xts[b], "wkq": wkq_sb, "wv": wv_sb})
    return maps


def _install_profile_shim():
    import sys, types
    import concourse.bass_utils as bu
    bu.upload_artifacts = lambda tmpdir: "local://" + tmpdir
    if "antenv.axon_hooks" in sys.modules:
        return
    mod = types.ModuleType("antenv.axon_hooks")
    holder = []
    mod.set_axon_ntff_profile_hook = holder.append
    mod.get_axon_ntff_profile_hook = lambda: holder[-1] if holder else None
    sys.modules["antenv.axon_hooks"] = mod
    import antenv
    antenv.axon_hooks = mod
    from trn_agent_boot.trn_boot import _ntff_profile_via_ctypes
    mod.set_axon_ntff_profile_hook(_ntff_profile_via_ctypes("/opt/axon/libaxon_pjrt.so"))


def kernel(x, Wq, Wk, Wv, _want_profile=False):
    if _want_profile:
        _install_profile_shim()
    nc = _build()
    maps = _in_maps(x, Wq, Wk, Wv)
    res = run_bass_kernel_spmd(nc, maps, core_ids=list(range(NCORES)),
                               trace=_want_profile)
    out = np.empty((B, T, H), np.float32)
    for c in range(NCORES):
        b, half = c % 4, c // 4
        r = np.asarray(res.results[c]["out"])  # [64, 2048]
        if half == 0:
            out[b, 0:1024] = r[:, 0:1024].T
            out[b, 3072:4096] = r[:, 1024:2048].T
        else:
            out[b, 1024:3072] = r.T
    if _want_profile:
        return out, res
    return out


# revision 19
# speedup vs baseline: 1.3471x; 1.0982x over previous
"""Causal single-head attention (B=4, T=4096, D=1024, H=64) on 8 TRN2 cores.

Sharding: 2 cores per batch, queries split zig-zag for causal load balance:
  half0 (cores 0-3):  query rows [0,1024) u [3072,4096), keys all [0,4096)
  half1 (cores 4-7):  query rows [1024,3072),             keys     [0,3072)

The host pre-transposes and bf16-casts x per core (x^T [D, T]) so the
kernel needs NO on-device transpose of x: projections read x^T directly
(d on partitions).  Weights are host-packed into SBUF partition layout.
HBM traffic per core: 8.4MB (bf16) instead of 16.8MB (f32).

Phase 1 (uniform, outside the If): for each 512-row group g:
  pqk[128, 512] = [Wk|Wq]^T x^T   (8 accumulating matmuls over d-chunks)
  pv[64, 512]   = Wv^T x^T        -> vT -> PE-transpose -> vsb [s, h|1]
  kT/qT partition dup via SBUF-SBUF DMA at the end (for row-packed scores).

Phase 2 (If/Else on partition id, engines PE/Act/DVE only):
  scores^T[s,t] built per 128-key chunk (row-packed concurrent pairs),
  exp on Act engine (no max subtraction; scores*scale bounded ~+-2.5),
  causal mask multiply on DVE for diagonal chunks,
  PV accumulation with ones-column (m=65) giving the denominator,
  epilogue: DVE reciprocal of denom row + 1-row broadcast matmul + DVE
  multiply; output stays transposed [64, 2048] and the host transposes.
"""

import numpy as np
import ml_dtypes

import concourse.bass as bass
import concourse.mybir as mybir
from concourse import bacc
from concourse.tile import TileContext
from concourse.masks import make_identity
from concourse.bass_utils import run_bass_kernel_spmd

B, T, D, H = 4, 4096, 1024, 64
NCORES = 8
NQ = 2048
SCALE = 1.0 / np.sqrt(D)  # 1/32
BF16 = ml_dtypes.bfloat16

# per-half schedules: list of (out_slot, q_tile, [chunks, fills first then
# diag]) emitted biggest-tile-first so DVE epilogues hide under PE chains
def _sched(half):
    tiles = [0, 1, 6, 7] if half == 0 else [2, 3, 4, 5]
    out = []
    for slot, t in enumerate(tiles):
        diag = [4 * t + i for i in range(4)]
        fills = list(range(0, 4 * t))
        out.append((slot, t, fills + diag))
    return out[::-1]

_CACHE = {}


def _build():
    if "nc" in _CACHE:
        return _CACHE["nc"]
    f32 = mybir.dt.float32
    bf16 = mybir.dt.bfloat16
    AF = mybir.ActivationFunctionType

    nc = bacc.Bacc(None, target_bir_lowering=False)
    # x^T in group-major layout: x_d[g, p, c*512+t'] = x[512g+t', 128c+p]
    # -> per-(g,p) the DMA reads one contiguous 8KB run
    x_d = nc.declare_dram_parameter("xt", [8, 128, 4096], bf16, isOutput=False)
    wkq_d = nc.declare_dram_parameter("wkq", [128, 1024], bf16, isOutput=False)
    wv_d = nc.declare_dram_parameter("wv", [128, 512], bf16, isOutput=False)
    out_d = nc.declare_dram_parameter("out", [H, NQ], f32, isOutput=True)

    with TileContext(nc) as tc:
        with (
            tc.tile_pool(name="persist", bufs=1) as pp,
            tc.tile_pool(name="work", bufs=2) as pw,
        ):
            # ---- weights (sync) + x^T groups (group 0 first) ----
            wkq = pp.tile([128, 1024], bf16, tag="wkq")
            nc.sync.dma_start(out=wkq[:, :], in_=wkq_d[:, :])
            wv = pp.tile([128, 512], bf16, tag="wv")
            nc.sync.dma_start(out=wv[:, :], in_=wv_d[:, :])

            xsb = pp.tile([128, 8 * T], bf16, tag="xsb")  # [p, (dc, t)]
            xview = xsb[:, :].rearrange("p (c t) -> p c t", t=T)

            def xload(eng, g):
                eng.dma_start(
                    out=xview[:, :, 512 * g: 512 * (g + 1)],
                    in_=x_d[g, :, :].rearrange("p (c t) -> p c t", t=512))

            for g in range(8):
                xload(nc.gpsimd if g < 4 else nc.sync, g)

            # ---- constants ----
            ident_f = pp.tile([128, 128], f32, tag="idf")
            make_identity(nc, ident_f[:, :])
            ident_b = pp.tile([128, 128], bf16, tag="idb")
            nc.vector.tensor_copy(ident_b[:, :], ident_f[:, :])

            # mask_big[p, g] = 1 iff g >= p + 384 (else 0)
            mask_f = pp.tile([128, 896], f32, tag="mkf")
            nc.gpsimd.memset(mask_f[:, :], 0.0)
            nc.gpsimd.affine_select(
                out=mask_f[:, :], in_=mask_f[:, :],
                compare_op=mybir.AluOpType.is_gt, fill=1.0,
                base=384, pattern=[[-1, 896]], channel_multiplier=1,
            )
            mask_b = pp.tile([128, 896], bf16, tag="mkb")
            nc.vector.tensor_copy(mask_b[:, :], mask_f[:, :])

            # persistent activations
            kT = pp.tile([128, T], bf16, tag="kT")     # k^T; rows 64:128 dup
            qT = pp.tile([128, T], bf16, tag="qT")     # q^T; rows 0:64 dup
            # vsb chunk ch: cols 0:64 = v rows of key chunk ch, cols 64:128 = 1
            # -> PV matmul (m=128) yields numerator on psum rows 0:64 and the
            #    denominator REPLICATED on rows 64:128 (same cycle count).
            vsb = pp.tile([128, 32 * 128], bf16, tag="vsb")
            nc.vector.memset(vsb[:, :], 1.0)

            # preload the exp activation table early (hide the ~1.3us load)
            warm = pw.tile([1, 1], f32, tag="warm")
            nc.scalar.activation(warm[:, :], mask_b[0:1, 0:1], AF.Exp, scale=1.0)

            # ---- phase 1: project, 8 groups of 512 t-columns ----
            with tc.tile_pool(name="ps1", bufs=2, space="PSUM") as ps1:
                for g in range(8):
                    tsl = slice(512 * g, 512 * (g + 1))
                    pqk = ps1.tile([128, 512], f32, tag="pqk")
                    for dc in range(8):
                        nc.tensor.matmul(
                            pqk[:, :], lhsT=wkq[:, 128 * dc: 128 * (dc + 1)],
                            rhs=xsb[:, 4096 * dc + 512 * g: 4096 * dc + 512 * (g + 1)],
                            start=(dc == 0), stop=(dc == 7))
                    nc.scalar.copy(kT[0:64, tsl], pqk[0:64, :])
                    nc.vector.tensor_copy(qT[64:128, tsl], pqk[64:128, :])

                    pv = ps1.tile([64, 512], f32, tag="pv")
                    for dc in range(8):
                        nc.tensor.matmul(
                            pv[:, :], lhsT=wv[:, 64 * dc: 64 * (dc + 1)],
                            rhs=xsb[:, 4096 * dc + 512 * g: 4096 * dc + 512 * (g + 1)],
                            start=(dc == 0), stop=(dc == 7))
                    vT = pw.tile([64, 512], bf16, tag="vT")
                    nc.scalar.copy(vT[:, :], pv[:, :])
                    pvn = ps1.tile([128, 256], f32, tag="pvn")
                    for c in range(4):
                        nc.tensor.matmul(
                            pvn[:, 64 * c: 64 * (c + 1)],
                            lhsT=vT[0:64, 128 * c: 128 * (c + 1)],
                            rhs=ident_b[0:64, 0:64], start=True, stop=True)
                    nc.vector.tensor_copy(
                        vsb[:, 128 * 4 * g: 128 * 4 * (g + 1)].rearrange(
                            "p (c h) -> p c h", h=128)[:, :, 0:64],
                        pvn[:, :].rearrange("p (c h) -> p c h", h=64))

            # partition dups: kT rows 0:64 -> 64:128, qT rows 64:128 -> 0:64
            for hg in range(2):
                csl = slice(2048 * hg, 2048 * (hg + 1))
                nc.sync.dma_start(out=kT[64:128, csl], in_=kT[0:64, csl])
                nc.sync.dma_start(out=qT[0:64, csl], in_=qT[64:128, csl])

            # ---- phase 2: attention, specialized per half ----
            with (
                tc.tile_pool(name="ps2", bufs=1, space="PSUM") as ps2,
                tc.tile_pool(name="ps3", bufs=1, space="PSUM") as ps3,
            ):
                # all tiles pre-allocated OUTSIDE the If (pool allocation
                # inside conditional branches breaks Tile's wait assignment)
                ps_bufs = [ps2.tile([128, 1024], f32, tag=f"sc{i}", name=f"scb{i}") for i in range(3)]
                pT_bufs = [pw.tile([128, 1024], bf16, tag=f"pT{i}", name=f"pTb{i}") for i in range(3)]
                po_bufs = [ps3.tile([128, 512], f32, tag=f"po{i}", name=f"pob{i}") for i in range(2)]
                rc_bufs = [pw.tile([128, 512], f32, tag=f"rc{i}", name=f"rcb{i}") for i in range(2)]
                outsb = pw.tile([64, 2048], f32, tag="outsb")

                def phase2(half):
                    # flat pair list: (slot, qt, c0, c1, pos, n)
                    pairs = []
                    for slot, qt, chunks in _sched(half):
                        n = len(chunks)  # multiple of 4
                        for pos in range(0, n, 2):
                            pairs.append((slot, qt, chunks[pos], chunks[pos + 1], pos, n))

                    def emit_scores(i):
                        slot, qt, c0, c1, pos, n = pairs[i]
                        tsl = slice(512 * qt, 512 * (qt + 1))
                        ps = ps_bufs[i % 3]
                        nc.tensor.matmul(
                            ps[:, 0:512],
                            lhsT=kT[0:64, 128 * c0: 128 * (c0 + 1)],
                            rhs=qT[0:64, tsl], start=True, stop=True)
                        nc.tensor.matmul(
                            ps[:, 512:1024],
                            lhsT=kT[64:128, 128 * c1: 128 * (c1 + 1)],
                            rhs=qT[64:128, tsl], start=True, stop=True)

                    emit_scores(0)
                    for i, (slot, qt, c0, c1, pos, n) in enumerate(pairs):
                        ps = ps_bufs[i % 3]
                        pT = pT_bufs[i % 3]
                        po = po_bufs[slot % 2]
                        nc.scalar.activation(pT[:, :], ps[:, :], AF.Exp, scale=SCALE)
                        # keep PE busy during exp(i): emit next pair's scores
                        if i + 1 < len(pairs):
                            emit_scores(i + 1)
                        for jj, ch in enumerate((c0, c1)):
                            p = pos + jj
                            if p >= n - 4:  # diagonal chunk: causal mask
                                delta = 128 * (p - (n - 4))
                                nc.vector.tensor_mul(
                                    pT[:, 512 * jj: 512 * (jj + 1)],
                                    pT[:, 512 * jj: 512 * (jj + 1)],
                                    mask_b[:, 384 - delta: 896 - delta])
                            nc.tensor.matmul(
                                po[:, :], lhsT=vsb[:, 128 * ch: 128 * (ch + 1)],
                                rhs=pT[:, 512 * jj: 512 * (jj + 1)],
                                start=(p == 0), stop=(p == n - 1))
                        if pos + 2 == n:
                            # epilogue: denominator arrives replicated on psum
                            # rows 64:128; shifted reciprocal then multiply
                            rc = rc_bufs[slot % 2]
                            nc.vector.reciprocal(rc[0:64, :], po[64:128, :])
                            nc.vector.tensor_mul(
                                outsb[:, 512 * slot: 512 * (slot + 1)],
                                po[0:64, :], rc[0:64, :])

                pid = nc.partition_id(engines=[
                    mybir.EngineType.PE, mybir.EngineType.Activation,
                    mybir.EngineType.DVE])
                with tc.If(pid < 4) as cmp:
                    phase2(0)
                with cmp.Else():
                    phase2(1)

                # store after the If, sliced per slot so early slots overlap
                # later compute (emission order is slot 3,2,1,0)
                for slot in (3, 2, 1, 0):
                    nc.sync.dma_start(
                        out=out_d[:, 512 * slot: 512 * (slot + 1)],
                        in_=outsb[:, 512 * slot: 512 * (slot + 1)])

    nc.compile()
    _CACHE["nc"] = nc
    return nc


def _in_maps(x, Wq, Wk, Wv):
    wkq = np.concatenate([np.asarray(Wk), np.asarray(Wq)], axis=1).astype(BF16)
    wkq_sb = np.ascontiguousarray(
        wkq.reshape(8, 128, 128).transpose(1, 0, 2).reshape(128, 1024))
    wv_sb = np.ascontiguousarray(
        np.asarray(Wv).astype(BF16).reshape(8, 128, 64)
        .transpose(1, 0, 2).reshape(128, 512))
    # X5[g, p, c, t'] = x[512g+t', 128c+p]  (8KB contiguous per (g, p))
    xts = [np.ascontiguousarray(
        np.asarray(x[b]).astype(BF16).reshape(8, 512, 8, 128)
        .transpose(0, 3, 2, 1).reshape(8, 128, 4096)) for b in range(B)]
    maps = []
    for c in range(NCORES):
        b = c % 4
        maps.append({"xt": xts[b], "wkq": wkq_sb, "wv": wv_sb})
    return maps


def _install_profile_shim():
    import sys, types
    import concourse.bass_utils as bu
    bu.upload_artifacts = lambda tmpdir: "local://" + tmpdir
    if "antenv.axon_hooks" in sys.modules:
        return
    mod = types.ModuleType("antenv.axon_hooks")
    holder = []
    mod.set_axon_ntff_profile_hook = holder.append
    mod.get_axon_ntff_profile_hook = lambda: holder[-1] if holder else None
    sys.modules["antenv.axon_hooks"] = mod
    import antenv
    antenv.axon_hooks = mod
    from trn_agent_boot.trn_boot import _ntff_profile_via_ctypes
    mod.set_axon_ntff_profile_hook(_ntff_profile_via_ctypes("/opt/axon/libaxon_pjrt.so"))


def kernel(x, Wq, Wk, Wv, _want_profile=False):
    if _want_profile:
        _install_profile_shim()
    nc = _build()
    maps = _in_maps(x, Wq, Wk, Wv)
    res = run_bass_kernel_spmd(nc, maps, core_ids=list(range(NCORES)),
                               trace=_want_profile)
    out = np.empty((B, T, H), np.float32)
    for c in range(NCORES):
        b, half = c % 4, c // 4
        r = np.asarray(res.results[c]["out"])  # [64, 2048]
        if half == 0:
            out[b, 0:1024] = r[:, 0:1024].T
            out[b, 3072:4096] = r[:, 1024:2048].T
        else:
            out[b, 1024:3072] = r.T
    if _want_profile:
        return out, res
    return out


# revision 22
# speedup vs baseline: 1.3535x; 1.0047x over previous
"""Causal single-head attention (B=4, T=4096, D=1024, H=64) on 8 TRN2 cores.

Sharding: 2 cores per batch; queries split for causal load balance:
  half0 (cores 0-3):  query tiles {0,3,4,7} (x512 rows), keys [0,4096)
  half1 (cores 4-7):  query tiles {1,2,5,6},              keys [0,3584)
Both halves own 72 key chunks of attention work; tiles become ready
right after their own projection group, spreading attention through the
projection stream (tile t only needs key groups <= t and its own q).

The host pre-transposes and bf16-casts x per core (x^T, group-major so
every (group, partition) DMA line is one contiguous 8KB run).  Weights
are host-packed into SBUF partition layout: [Wk|Wq], [Wv], [Wk|Wv].
HBM traffic per core: 8.4MB bf16 instead of 16.8MB f32.

One fully-specialized If/Else (engines PE/Act/DVE); everything except
DMAs runs inside the branch:
  per group g: pqk[128,512] = [Wk|Wq]^T x^T (q-groups; plus a Wv pass)
               or [Wk|Wv]^T x^T (kv-only groups, single m=128 pass)
  partition dups for row-packed score pairs via partition-shifted
  Act/DVE copies straight out of PSUM (no DMA, no extra PE work).
  attention per tile: scores^T per 128-key chunk (row-packed concurrent
  pairs, one pair per [128,1024] psum group), exp on Act (no max
  subtraction; scores*scale bounded ~+-2.5), causal mask multiply on
  DVE for the 4 diagonal chunks (processed last), PV accumulation with
  vsb chunks [v | ones] (m=128) so the softmax denominator lands
  replicated on psum rows 64:128: epilogue = partition-shifted DVE
  reciprocal + aligned multiply.  Output stays transposed [64, 2048];
  the host transposes back during unsharding.
"""

import numpy as np
import ml_dtypes

import concourse.bass as bass
import concourse.mybir as mybir
from concourse import bacc
from concourse.tile import TileContext
from concourse.masks import make_identity
from concourse.bass_utils import run_bass_kernel_spmd

B, T, D, H = 4, 4096, 1024, 64
NCORES = 8
NQ = 2048
SCALE = 1.0 / np.sqrt(D)  # 1/32
BF16 = ml_dtypes.bfloat16

QTILES = {0: [0, 3, 4, 7], 1: [1, 2, 5, 6]}
NGROUPS = {0: 8, 1: 7}

_CACHE = {}


def _build():
    if "nc" in _CACHE:
        return _CACHE["nc"]
    f32 = mybir.dt.float32
    bf16 = mybir.dt.bfloat16
    AF = mybir.ActivationFunctionType

    nc = bacc.Bacc(None, target_bir_lowering=False)
    # x^T in group-major layout: x_d[g, p, c*512+t'] = x[512g+t', 128c+p]
    x_d = nc.declare_dram_parameter("xt", [8, 128, 4096], bf16, isOutput=False)
    wkq_d = nc.declare_dram_parameter("wkq", [128, 1024], bf16, isOutput=False)
    wv_d = nc.declare_dram_parameter("wv", [128, 512], bf16, isOutput=False)
    wkv_d = nc.declare_dram_parameter("wkv", [128, 1024], bf16, isOutput=False)
    out_d = nc.declare_dram_parameter("out", [H, NQ], f32, isOutput=True)

    with TileContext(nc) as tc:
        with (
            tc.tile_pool(name="persist", bufs=1) as pp,
            tc.tile_pool(name="work", bufs=2) as pw,
        ):
            # ---- weights (sync) + x^T groups (group 0 first on gpsimd) ----
            wkq = pp.tile([128, 1024], bf16, tag="wkq")
            nc.sync.dma_start(out=wkq[:, :], in_=wkq_d[:, :])
            wv = pp.tile([128, 512], bf16, tag="wv")
            nc.sync.dma_start(out=wv[:, :], in_=wv_d[:, :])
            wkv = pp.tile([128, 1024], bf16, tag="wkv")
            nc.sync.dma_start(out=wkv[:, :], in_=wkv_d[:, :])

            xsb = pp.tile([128, 8 * T], bf16, tag="xsb")  # [p, (dc, t)]
            xview = xsb[:, :].rearrange("p (c t) -> p c t", t=T)
            for g in range(8):
                eng = nc.gpsimd if g < 4 else nc.sync
                eng.dma_start(
                    out=xview[:, :, 512 * g: 512 * (g + 1)],
                    in_=x_d[g, :, :].rearrange("p (c t) -> p c t", t=512))

            # ---- constants ----
            ident_f = pp.tile([128, 128], f32, tag="idf")
            make_identity(nc, ident_f[:, :])
            ident_b = pp.tile([128, 128], bf16, tag="idb")
            nc.vector.tensor_copy(ident_b[:, :], ident_f[:, :])

            # mask_big[p, g] = 1 iff g >= p + 384 (else 0)
            mask_f = pp.tile([128, 896], f32, tag="mkf")
            nc.gpsimd.memset(mask_f[:, :], 0.0)
            nc.gpsimd.affine_select(
                out=mask_f[:, :], in_=mask_f[:, :],
                compare_op=mybir.AluOpType.is_gt, fill=1.0,
                base=384, pattern=[[-1, 896]], channel_multiplier=1,
            )
            mask_b = pp.tile([128, 896], bf16, tag="mkb")
            nc.vector.tensor_copy(mask_b[:, :], mask_f[:, :])

            # persistent activations
            kT = pp.tile([128, T], bf16, tag="kT")     # k^T; rows 64:128 dup
            qT = pp.tile([128, T], bf16, tag="qT")     # q^T; rows 0:64 dup
            # vsb chunk ch: cols 0:64 = v rows of key chunk ch, cols 64:128 = 1
            # -> PV matmul (m=128) yields numerator on psum rows 0:64 and the
            #    denominator replicated on rows 64:128 (same cycle count)
            vsb = pp.tile([128, 32 * 128], bf16, tag="vsb")
            nc.vector.memset(vsb[:, :], 1.0)

            # preload the exp activation table early (hide the ~1.3us load)
            warm = pw.tile([1, 1], f32, tag="warm")
            nc.scalar.activation(warm[:, :], mask_b[0:1, 0:1], AF.Exp, scale=1.0)

            with (
                tc.tile_pool(name="psA", bufs=1, space="PSUM") as psA,
                tc.tile_pool(name="ps2", bufs=1, space="PSUM") as ps2,
                tc.tile_pool(name="ps3", bufs=1, space="PSUM") as ps3,
            ):
                # everything pre-allocated OUTSIDE the If (pool allocation
                # inside conditional branches breaks Tile's wait assignment)
                pjA = psA.tile([128, 512], f32, tag="pjA")   # [k|q] / [k|v]
                pjB = psA.tile([128, 512], f32, tag="pjB")   # v pass + vn out
                ps_bufs = [ps2.tile([128, 1024], f32, tag=f"sc{i}", name=f"scb{i}") for i in range(2)]
                pT_bufs = [pw.tile([128, 1024], bf16, tag=f"pT{i}", name=f"pTb{i}") for i in range(3)]
                po_bufs = [ps3.tile([128, 512], f32, tag=f"po{i}", name=f"pob{i}") for i in range(2)]
                rc_bufs = [pw.tile([128, 512], f32, tag=f"rc{i}", name=f"rcb{i}") for i in range(2)]
                vT_bufs = [pw.tile([64, 512], bf16, tag=f"vT{i}", name=f"vTb{i}") for i in range(2)]
                outsb = pw.tile([64, 2048], f32, tag="outsb")

                def emit_proj(g, is_q):
                    tsl = slice(512 * g, 512 * (g + 1))

                    def xrhs(dc):
                        return xsb[:, 4096 * dc + 512 * g: 4096 * dc + 512 * (g + 1)]

                    pqk = pjA
                    w0 = wkq if is_q else wkv
                    for dc in range(8):
                        nc.tensor.matmul(
                            pqk[:, :], lhsT=w0[:, 128 * dc: 128 * (dc + 1)],
                            rhs=xrhs(dc), start=(dc == 0), stop=(dc == 7))
                    nc.scalar.copy(kT[0:64, tsl], pqk[0:64, :])
                    nc.vector.tensor_copy(kT[64:128, tsl], pqk[0:64, :])
                    vT = vT_bufs[g % 2]
                    if is_q:
                        nc.scalar.copy(qT[0:64, tsl], pqk[64:128, :])
                        nc.vector.tensor_copy(qT[64:128, tsl], pqk[64:128, :])
                        for dc in range(8):
                            nc.tensor.matmul(
                                pjB[0:64, :], lhsT=wv[:, 64 * dc: 64 * (dc + 1)],
                                rhs=xrhs(dc), start=(dc == 0), stop=(dc == 7))
                        nc.scalar.copy(vT[:, :], pjB[0:64, :])
                    else:
                        nc.scalar.copy(vT[:, :], pqk[64:128, :])
                    for c in range(4):
                        nc.tensor.matmul(
                            pjB[:, 64 * c: 64 * (c + 1)],
                            lhsT=vT[0:64, 128 * c: 128 * (c + 1)],
                            rhs=ident_b[0:64, 0:64], start=True, stop=True)
                    nc.vector.tensor_copy(
                        vsb[:, 512 * g: 512 * (g + 1)].rearrange(
                            "p (c h) -> p c h", h=128)[:, :, 0:64],
                        pjB[:, 0:256].rearrange("p (c h) -> p c h", h=64))

                def emit_attn(t, slot, gi0):
                    # chunks: fills first, diagonal (masked) last
                    chunks = list(range(0, 4 * t)) + [4 * t + i for i in range(4)]
                    n = len(chunks)
                    tsl = slice(512 * t, 512 * (t + 1))
                    po = po_bufs[slot % 2]

                    def emit_scores(i):
                        c0, c1 = chunks[2 * i], chunks[2 * i + 1]
                        ps = ps_bufs[(gi0 + i) % 2]
                        nc.tensor.matmul(
                            ps[:, 0:512],
                            lhsT=kT[0:64, 128 * c0: 128 * (c0 + 1)],
                            rhs=qT[0:64, tsl], start=True, stop=True)
                        nc.tensor.matmul(
                            ps[:, 512:1024],
                            lhsT=kT[64:128, 128 * c1: 128 * (c1 + 1)],
                            rhs=qT[64:128, tsl], start=True, stop=True)

                    npairs = n // 2
                    emit_scores(0)
                    for i in range(npairs):
                        ps = ps_bufs[(gi0 + i) % 2]
                        pT = pT_bufs[(gi0 + i) % 3]
                        nc.scalar.activation(pT[:, :], ps[:, :], AF.Exp, scale=SCALE)
                        # keep PE busy during exp(i)
                        if i + 1 < npairs:
                            emit_scores(i + 1)
                        for jj in range(2):
                            p = 2 * i + jj
                            ch = chunks[p]
                            if p >= n - 4:  # diagonal chunk: causal mask
                                delta = 128 * (p - (n - 4))
                                nc.vector.tensor_mul(
                                    pT[:, 512 * jj: 512 * (jj + 1)],
                                    pT[:, 512 * jj: 512 * (jj + 1)],
                                    mask_b[:, 384 - delta: 896 - delta])
                            nc.tensor.matmul(
                                po[:, :], lhsT=vsb[:, 128 * ch: 128 * (ch + 1)],
                                rhs=pT[:, 512 * jj: 512 * (jj + 1)],
                                start=(p == 0), stop=(p == n - 1))
                    # epilogue: shifted reciprocal of the replicated
                    # denominator, then aligned multiply
                    rc = rc_bufs[slot % 2]
                    nc.vector.reciprocal(rc[0:64, :], po[64:128, :])
                    nc.vector.tensor_mul(
                        outsb[:, 512 * slot: 512 * (slot + 1)],
                        po[0:64, :], rc[0:64, :])
                    return gi0 + npairs

                def body(half):
                    qtiles = QTILES[half]
                    slot_of = {t: i for i, t in enumerate(qtiles)}
                    gi = 0
                    for g in range(NGROUPS[half]):
                        emit_proj(g, g in slot_of)
                        if g in slot_of:
                            gi = emit_attn(g, slot_of[g], gi)

                pid = nc.partition_id(engines=[
                    mybir.EngineType.PE, mybir.EngineType.Activation,
                    mybir.EngineType.DVE])
                with tc.If(pid < 4) as cmp:
                    body(0)
                with cmp.Else():
                    body(1)

                # store after the If, sliced per slot so early slots overlap
                for slot in range(4):
                    nc.sync.dma_start(
                        out=out_d[:, 512 * slot: 512 * (slot + 1)],
                        in_=outsb[:, 512 * slot: 512 * (slot + 1)])

    nc.compile()
    _CACHE["nc"] = nc
    return nc


def _in_maps(x, Wq, Wk, Wv):
    def pack128(w):  # [1024, m] -> [128, 8*m] partition layout
        m = w.shape[1]
        return np.ascontiguousarray(
            w.astype(BF16).reshape(8, 128, m).transpose(1, 0, 2).reshape(128, 8 * m))

    Wk, Wq, Wv = (np.asarray(a) for a in (Wk, Wq, Wv))
    wkq_sb = pack128(np.concatenate([Wk, Wq], axis=1))
    wv_sb = pack128(Wv)
    wkv_sb = pack128(np.concatenate([Wk, Wv], axis=1))
    # X5[g, p, c, t'] = x[512g+t', 128c+p]  (8KB contiguous per (g, p))
    xts = [np.ascontiguousarray(
        np.asarray(x[b]).astype(BF16).reshape(8, 512, 8, 128)
        .transpose(0, 3, 2, 1).reshape(8, 128, 4096)) for b in range(B)]
    maps = []
    for c in range(NCORES):
        b = c % 4
        maps.append({"xt": xts[b], "wkq": wkq_sb, "wv": wv_sb, "wkv": wkv_sb})
    return maps


def _install_profile_shim():
    import sys, types
    import concourse.bass_utils as bu
    bu.upload_artifacts = lambda tmpdir: "local://" + tmpdir
    if "antenv.axon_hooks" in sys.modules:
        return
    mod = types.ModuleType("antenv.axon_hooks")
    holder = []
    mod.set_axon_ntff_profile_hook = holder.append
    mod.get_axon_ntff_profile_hook = lambda: holder[-1] if holder else None
    sys.modules["antenv.axon_hooks"] = mod
    import antenv
    antenv.axon_hooks = mod
    from trn_agent_boot.trn_boot import _ntff_profile_via_ctypes
    mod.set_axon_ntff_profile_hook(_ntff_profile_via_ctypes("/opt/axon/libaxon_pjrt.so"))


def kernel(x, Wq, Wk, Wv, _want_profile=False):
    if _want_profile:
        _install_profile_shim()
    nc = _build()
    maps = _in_maps(x, Wq, Wk, Wv)
    res = run_bass_kernel_spmd(nc, maps, core_ids=list(range(NCORES)),
                               trace=_want_profile)
    out = np.empty((B, T, H), np.float32)
    for c in range(NCORES):
        b, half = c % 4, c // 4
        r = np.asarray(res.results[c]["out"])  # [64, 2048]
        for slot, t in enumerate(QTILES[half]):
            out[b, 512 * t: 512 * (t + 1)] = r[:, 512 * slot: 512 * (slot + 1)].T
    if _want_profile:
        return out, res
    return out
